# revision 6
# baseline (speedup 1.0000x reference)
"""Expert-parallel grouped MLP (MoE routing) for Trainium2.

Problem: x[16384,1024] fp32, w1[8,1024,4096], w2[8,4096,1024],
rows_per_expert=2048.  out = gelu(x_e @ w1[e]) @ w2[e] per expert group.

Sharding: one expert per NeuronCore (E=8 == n_cores).  Each core runs an
identical Bass program on its own expert's slice; no collectives.  The host
pre-permutes each operand so every DMA chunk is a fully contiguous DRAM
region with 2-8KB per-partition lines:
    x  -> [NBLK, 128, HO, T_BLK]   (xp[b,p,h,ti]  = x[b*T_BLK+ti, h*128+p])
    w1 -> [FO, 128, H]             (w1p[f,p,h*128+fi] = w1[h*128+p, f*128+fi])
    w2 -> [HO, 128, F]             (w2p[h,p,f*128+hi] = w2[f*128+p, h*128+hi])
    out <- [NBLK, HO, 128, T_BLK]  (out4[b,h,p,ti] = out[b*T_BLK+ti, h*128+p])
Activations stay in [feature, token] orientation through both GEMMs:
    GEMM1: interT[f,t] = sum_h w1[h,f] * xT[h,t]    (lhsT = w1 tile)
    gelu on PSUM -> SBUF (bf16)
    GEMM2: outT[h,t]  = sum_f w2[f,h] * interT[f,t]  (lhsT = w2 tile)
Matmuls run in bf16 (fp32 PSUM accumulate) - fp32 matmul is 4x slower on
the PE array.  Weights are SBUF-resident (64KB/partition each); tokens are
processed in 4 blocks of 512 so interT fits in SBUF.  Weight-chunk DMAs are
gated on compute progress (only ~1.3MB gates the first matmul) and dummy
matmuls on a zeroed tile warm the PE clock (HAM) during the initial DMA.
"""

import numpy as np
import ml_dtypes

E = 8
H = 1024
F = 4096
T_PER_E = 2048
T_BLK = 512
NBLK = T_PER_E // T_BLK
P = 128
HO = H // P    # 8 contraction chunks for GEMM1
FO = F // P    # 32 contraction chunks for GEMM2
NW2 = 8        # w2 staged in HO chunks
NWARM = 12     # PE warm-up matmuls (N=256, cold ~213ns each)
W1_UNGATED = 5     # leading w1 chunks that stream immediately
W1_LOOKAHEAD = 5   # f-tiles of slack between a w1 chunk's DMA gate and its use

TRACE = False          # test.py sets kernel.TRACE = True for profiling
LAST_RESULTS = None    # BassKernelResults of the most recent run

_nc_cache = None


def _build_nc():
    import concourse.mybir as mybir
    import concourse.tile as tile
    from concourse import bacc
    from concourse.tile_rust import add_dep_helper

    bf16 = mybir.dt.bfloat16
    f32 = mybir.dt.float32
    GELU = mybir.ActivationFunctionType.Gelu_apprx_tanh

    nc = bacc.Bacc("TRN2", target_bir_lowering=False, debug=False)

    xp = nc.dram_tensor("xp", [NBLK, P, HO, T_BLK], bf16, kind="ExternalInput").ap()
    w1p = nc.dram_tensor("w1p", [FO, P, H], bf16, kind="ExternalInput").ap()
    w2p = nc.dram_tensor("w2p", [HO, P, F], bf16, kind="ExternalInput").ap()
    # Output in bf16: halves the store traffic draining at the kernel tail;
    # the host upcasts to fp32.  The added rounding (~1e-3 relative, on top
    # of the ~3.4e-3 from the bf16 matmuls) is negligible.
    out4 = nc.dram_tensor("out4", [NBLK, HO, P, T_BLK], bf16, kind="ExternalOutput").ap()

    with tile.TileContext(nc) as tc:
        with (
            tc.tile_pool(name="wpool", bufs=1) as wpool,
            tc.tile_pool(name="xpool", bufs=2) as xpool,
            tc.tile_pool(name="ipool", bufs=1) as ipool,
            tc.tile_pool(name="opool", bufs=3) as opool,
            tc.tile_pool(name="ps1", bufs=4, space="PSUM") as ps1,
            tc.tile_pool(name="ps2", bufs=4, space="PSUM") as ps2,
        ):
            # PE warm-up: dummy matmuls keep the PE busy while the first real
            # operands stream in, so the HAM clock gate reaches full rate
            # before the first real matmul.  The memset runs on Vector, whose
            # engine preamble finishes ~1.2us before GpSimd's, so the first
            # warm matmul isn't chained behind the slow GpSimd startup.
            warm = wpool.tile([P, 256], bf16, tag="warm")
            nc.vector.memset(warm[:], 0.0)
            for _ in range(NWARM):
                wp = ps2.tile([P, T_BLK], f32, tag="ps2t")
                nc.tensor.matmul(wp[:, 0:256], warm[:, 0:P], warm[:], start=True, stop=True)

            # w1 layout [P, FO, H]: lhsT for (h,f) = w1_sb[:, f, h*128:(h+1)*128]
            # w2 layout [P, HO, F]: lhsT for (f,h) = w2_sb[:, h, f*128:(f+1)*128]
            w1_sb = wpool.tile([P, FO, H], bf16, tag="w1sb")
            w2_sb = wpool.tile([P, HO, F], bf16, tag="w2sb")

            # Each HWDGE trigger costs ~0.65us of sync-sequencer time AND the
            # early DMA bandwidth is trigger-rate-limited (a 128KB chunk per
            # 0.65us trigger caps the ramp at ~200GB/s vs the 378GB/s the
            # fabric sustains).  So the startup loads use few, fat triggers
            # in consumption order: w1 chunk 0 first (gates the first real
            # matmul together with xb0[h0:2]), then xb0 in three slices
            # matched to the f0 chain's h order, then the ungated w1 chunks.
            xb0 = xpool.tile([P, HO, T_BLK], bf16, tag="xb")
            w1_dmas = [nc.sync.dma_start(w1_sb[:, 0, :], w1p[0])]
            xb0_dmas = [
                nc.sync.dma_start(xb0[:, 0:2, :], xp[0, :, 0:2, :]),
                nc.sync.dma_start(xb0[:, 2:5, :], xp[0, :, 2:5, :]),
                nc.sync.dma_start(xb0[:, 5:8, :], xp[0, :, 5:8, :]),
            ]
            w1_dmas += [nc.sync.dma_start(w1_sb[:, f, :], w1p[f])
                        for f in range(1, FO)]
            w2_dmas = [
                nc.sync.dma_start(w2_sb[:, h, :], w2p[h]) for h in range(NW2)
            ]
            mm_first = {}  # (b, f) -> first matmul of that f-tile

            for b in range(NBLK):
                if b == 0:
                    xb = xb0
                else:
                    xb = xpool.tile([P, HO, T_BLK], bf16, tag="xb")
                    nc.sync.dma_start(xb[:], xp[b])

                it = ipool.tile([P, FO, T_BLK], bf16, tag="inter")
                for f in range(FO):
                    ps = ps1.tile([P, T_BLK], f32, tag="ps1t")
                    for h in range(HO):
                        mm = nc.tensor.matmul(
                            ps[:],
                            w1_sb[:, f, h * P:(h + 1) * P],
                            xb[:, h, :],
                            start=(h == 0),
                            stop=(h == HO - 1),
                        )
                        if h == 0:
                            mm_first[(b, f)] = mm
                    nc.scalar.activation(it[:, f, :], ps[:], GELU)

                HB = T_BLK // 2
                for h in range(HO):
                    if not (b == NBLK - 1 and h == HO - 1):
                        ps = ps2.tile([P, T_BLK], f32, tag="ps2t")
                        for f in range(FO):
                            nc.tensor.matmul(
                                ps[:],
                                w2_sb[:, h, f * P:(f + 1) * P],
                                it[:, f, :],
                                start=(f == 0),
                                stop=(f == FO - 1),
                            )
                        # Evict in two halves so the DMA store of the first
                        # half overlaps the copy of the second.
                        ob = opool.tile([P, T_BLK], bf16, tag="ob")
                        nc.vector.tensor_copy(ob[:, :HB], ps[:, :HB])
                        nc.sync.dma_start(out4[b, h, :, :HB], ob[:, :HB])
                        nc.vector.tensor_copy(ob[:, HB:], ps[:, HB:])
                        nc.sync.dma_start(out4[b, h, :, HB:], ob[:, HB:])
                    else:
                        # Very last h-tile: run it as two independent N=256
                        # accumulation chains over token halves (separate
                        # PSUM banks) so the first half's copy+store overlap
                        # the second half's matmuls -- only 64KB drains after
                        # the final matmul instead of 128KB.
                        psA = ps2.tile([P, T_BLK], f32, tag="ps2t")
                        psB = ps2.tile([P, T_BLK], f32, tag="ps2t")
                        ob = opool.tile([P, T_BLK], bf16, tag="ob")
                        for f in range(FO):
                            nc.tensor.matmul(
                                psA[:, 0:HB],
                                w2_sb[:, h, f * P:(f + 1) * P],
                                it[:, f, 0:HB],
                                start=(f == 0),
                                stop=(f == FO - 1),
                            )
                        nc.vector.tensor_copy(ob[:, :HB], psA[:, 0:HB])
                        nc.sync.dma_start(out4[b, h, :, :HB], ob[:, :HB])
                        for f in range(FO):
                            nc.tensor.matmul(
                                psB[:, 0:HB],
                                w2_sb[:, h, f * P:(f + 1) * P],
                                it[:, f, HB:],
                                start=(f == 0),
                                stop=(f == FO - 1),
                            )
                        nc.vector.tensor_copy(ob[:, HB:], psB[:, 0:HB])
                        nc.sync.dma_start(out4[b, h, :, HB:], ob[:, HB:])

            # Stage the weight stream behind compute progress so the bulk of
            # the 16MB of weights never contends with the critical path:
            # w1 f-tile chunk c waits for the f-tile W1_LOOKAHEAD tiles ahead
            # of its first consumer; w2 chunk c is gated on the tail f-tiles
            # of GEMM1 block 0 (w2 is first read ~55us in).
            for c in range(W1_UNGATED, FO):
                add_dep_helper(
                    w1_dmas[c].ins, mm_first[(0, c - W1_LOOKAHEAD)].ins,
                    sync=True, reason="stage w1 load behind compute",
                )
            for c in range(NW2):
                add_dep_helper(
                    w2_dmas[c].ins, mm_first[(0, FO - NW2 - 6 + c)].ins,
                    sync=True, reason="stage w2 load behind compute",
                )
    nc.compile()
    return nc


def _get_nc():
    global _nc_cache
    if _nc_cache is None:
        _nc_cache = _build_nc()
    return _nc_cache


def kernel(x, w1, w2, rows_per_expert):
    global LAST_RESULTS
    from concourse.bass_utils import run_bass_kernel_spmd

    x = np.asarray(x)
    w1 = np.asarray(w1)
    w2 = np.asarray(w2)
    rpe = int(rows_per_expert)
    assert x.shape == (E * rpe, H) and rpe == T_PER_E
    assert w1.shape == (E, H, F) and w2.shape == (E, F, H)

    bf16 = ml_dtypes.bfloat16
    in_maps = []
    for e in range(E):
        xe = x[e * rpe:(e + 1) * rpe].astype(bf16)      # [T, H]
        # [b*T_BLK+ti, ho*128+p] -> [b, p, ho, ti]
        xpm = np.ascontiguousarray(
            xe.reshape(NBLK, T_BLK, HO, P).transpose(0, 3, 2, 1)
        )
        # w1[ho*128+p, f*128+fi] -> [f, p, ho*128+fi]
        w1m = np.ascontiguousarray(
            w1[e].astype(bf16).reshape(HO, P, FO, P).transpose(2, 1, 0, 3)
        ).reshape(FO, P, H)
        # w2[fo*128+p, h*128+hi] -> [h, p, fo*128+hi]
        w2m = np.ascontiguousarray(
            w2[e].astype(bf16).reshape(FO, P, HO, P).transpose(2, 1, 0, 3)
        ).reshape(HO, P, F)
        in_maps.append({"xp": xpm, "w1p": w1m, "w2p": w2m})

    res = run_bass_kernel_spmd(_get_nc(), in_maps, list(range(E)), trace=TRACE)
    LAST_RESULTS = res

    out = np.empty((E * rpe, H), dtype=np.float32)
    for e in range(E):
        # [b, h, p, ti] -> [b*T_BLK+ti, h*128+p]
        o4 = res.results[e]["out4"].astype(np.float32)
        out[e * rpe:(e + 1) * rpe] = o4.transpose(0, 3, 1, 2).reshape(rpe, H)
    return out



# revision 11
# speedup vs baseline: 1.0089x; 1.0089x over previous
"""Expert-parallel grouped MLP (MoE routing) for Trainium2.

Problem: x[16384,1024] fp32, w1[8,1024,4096], w2[8,4096,1024],
rows_per_expert=2048.  out = gelu(x_e @ w1[e]) @ w2[e] per expert group.

Sharding: one expert per NeuronCore (E=8 == n_cores).  Each core runs an
identical Bass program on its own expert's slice; no collectives.  The host
pre-permutes each operand so every DMA chunk is a fully contiguous DRAM
region with 2-8KB per-partition lines:
    x  -> [NBLK, 128, HO, T_BLK]   (xp[b,p,h,ti]  = x[b*T_BLK+ti, h*128+p])
    w1 -> [FO, 128, H]             (w1p[f,p,h*128+fi] = w1[h*128+p, f*128+fi])
    w2 -> [HO, 128, F]             (w2p[h,p,f*128+hi] = w2[f*128+p, h*128+hi])
    out <- [NBLK, HO, 128, T_BLK]  (out4[b,h,p,ti] = out[b*T_BLK+ti, h*128+p])
Activations stay in [feature, token] orientation through both GEMMs:
    GEMM1: interT[f,t] = sum_h w1[h,f] * xT[h,t]    (lhsT = w1 tile)
    gelu on PSUM -> SBUF (bf16)
    GEMM2: outT[h,t]  = sum_f w2[f,h] * interT[f,t]  (lhsT = w2 tile)
Matmuls run in bf16 (fp32 PSUM accumulate) - fp32 matmul is 4x slower on
the PE array.  Weights are SBUF-resident (64KB/partition each); tokens are
processed in 4 blocks of 512 so interT fits in SBUF.  Weight-chunk DMAs are
gated on compute progress (only ~1.3MB gates the first matmul) and dummy
matmuls on a zeroed tile warm the PE clock (HAM) during the initial DMA.
"""

import numpy as np
import ml_dtypes

E = 8
H = 1024
F = 4096
T_PER_E = 2048
T_BLK = 512
NBLK = T_PER_E // T_BLK
P = 128
HO = H // P    # 8 contraction chunks for GEMM1
FO = F // P    # 32 contraction chunks for GEMM2
NW2 = 8        # w2 staged in HO chunks
NWARM = 21     # PE warm-up matmuls (N=256, cold ~213ns each)
W1_UNGATED = 5     # leading w1 chunks that stream immediately
W1_LOOKAHEAD = 5   # f-tiles of slack between a w1 chunk's DMA gate and its use

TRACE = False          # test.py sets kernel.TRACE = True for profiling
LAST_RESULTS = None    # BassKernelResults of the most recent run

_nc_cache = None


def _build_nc():
    import concourse.mybir as mybir
    import concourse.tile as tile
    from concourse import bacc
    from concourse.tile_rust import add_dep_helper

    bf16 = mybir.dt.bfloat16
    f32 = mybir.dt.float32
    GELU = mybir.ActivationFunctionType.Gelu_apprx_tanh

    nc = bacc.Bacc("TRN2", target_bir_lowering=False, debug=False)

    xp = nc.dram_tensor("xp", [NBLK, P, HO, T_BLK], bf16, kind="ExternalInput").ap()
    w1p = nc.dram_tensor("w1p", [FO, P, H], bf16, kind="ExternalInput").ap()
    w2p = nc.dram_tensor("w2p", [HO, P, F], bf16, kind="ExternalInput").ap()
    # Host-packed startup operands: per partition [w1 chunk 0 (H elems),
    # xb0 h0..h7 (HO*T_BLK elems)].  One fat 1.25MB DMA with 10KB lines runs
    # at full fabric rate from the first packet, where the baseline's nine
    # thin triggers were trigger-rate-limited to ~190GB/s during the ramp.
    BOOT = H + HO * T_BLK
    bootp = nc.dram_tensor("bootp", [P, BOOT], bf16, kind="ExternalInput").ap()
    # Output in bf16: halves the store traffic draining at the kernel tail;
    # the host upcasts to fp32.  The added rounding (~1e-3 relative, on top
    # of the ~3.4e-3 from the bf16 matmuls) is negligible.
    out4 = nc.dram_tensor("out4", [NBLK, HO, P, T_BLK], bf16, kind="ExternalOutput").ap()

    with tile.TileContext(nc) as tc:
        with (
            tc.tile_pool(name="wpool", bufs=1) as wpool,
            tc.tile_pool(name="xpool", bufs=2) as xpool,
            tc.tile_pool(name="ipool", bufs=1) as ipool,
            tc.tile_pool(name="opool", bufs=3) as opool,
            tc.tile_pool(name="ps1", bufs=4, space="PSUM") as ps1,
            tc.tile_pool(name="ps2", bufs=4, space="PSUM") as ps2,
        ):
            # PE warm-up: dummy matmuls keep the PE busy while the first real
            # operands stream in, so the HAM clock gate reaches full rate
            # before the first real matmul.  The memset runs on Vector, whose
            # engine preamble finishes ~1.2us before GpSimd's, so the first
            # warm matmul isn't chained behind the slow GpSimd startup.
            warm = wpool.tile([P, 256], bf16, tag="warm")
            nc.vector.memset(warm[:], 0.0)
            for _ in range(NWARM):
                wp = ps2.tile([P, T_BLK], f32, tag="ps2t")
                nc.tensor.matmul(wp[:, 0:256], warm[:, 0:P], warm[:], start=True, stop=True)

            # w1 layout [P, FO, H]: lhsT for (h,f) = w1_sb[:, f, h*128:(h+1)*128]
            # w2 layout [P, HO, F]: lhsT for (f,h) = w2_sb[:, h, f*128:(f+1)*128]
            w1_sb = wpool.tile([P, FO, H], bf16, tag="w1sb")
            w2_sb = wpool.tile([P, HO, F], bf16, tag="w2sb")

            # All block-0 GEMM1 inputs arrive in the single boot DMA; the
            # remaining w1 chunks stream one 256KB trigger each behind it.
            # Block 0 reads f=0 weights and its x tiles straight out of
            # boot_sb (w1_sb chunk 0 is never loaded; blocks 1-3 also take
            # their f=0 weights from boot_sb).
            boot_sb = wpool.tile([P, BOOT], bf16, tag="boot")
            w1_dmas = [nc.sync.dma_start(boot_sb[:], bootp)]
            w1_dmas += [nc.sync.dma_start(w1_sb[:, f, :], w1p[f])
                        for f in range(1, FO)]
            w2_dmas = [
                nc.sync.dma_start(w2_sb[:, h, :], w2p[h]) for h in range(NW2)
            ]
            mm_first = {}  # (b, f) -> first matmul of that f-tile

            def w1_tile(f, h):
                if f == 0:
                    return boot_sb[:, h * P:(h + 1) * P]
                return w1_sb[:, f, h * P:(h + 1) * P]

            for b in range(NBLK):
                if b == 0:
                    xb = None
                else:
                    xb = xpool.tile([P, HO, T_BLK], bf16, tag="xb")
                    nc.sync.dma_start(xb[:], xp[b])

                def x_tile(h, xb=xb):
                    if xb is None:
                        return boot_sb[:, H + h * T_BLK:H + (h + 1) * T_BLK]
                    return xb[:, h, :]

                it = ipool.tile([P, FO, T_BLK], bf16, tag="inter")
                for f in range(FO):
                    ps = ps1.tile([P, T_BLK], f32, tag="ps1t")
                    for h in range(HO):
                        mm = nc.tensor.matmul(
                            ps[:],
                            w1_tile(f, h),
                            x_tile(h),
                            start=(h == 0),
                            stop=(h == HO - 1),
                        )
                        if h == 0:
                            mm_first[(b, f)] = mm
                    nc.scalar.activation(it[:, f, :], ps[:], GELU)

                HB = T_BLK // 2
                for h in range(HO):
                    if not (b == NBLK - 1 and h == HO - 1):
                        ps = ps2.tile([P, T_BLK], f32, tag="ps2t")
                        for f in range(FO):
                            nc.tensor.matmul(
                                ps[:],
                                w2_sb[:, h, f * P:(f + 1) * P],
                                it[:, f, :],
                                start=(f == 0),
                                stop=(f == FO - 1),
                            )
                        # Evict in two halves so the DMA store of the first
                        # half overlaps the copy of the second.
                        ob = opool.tile([P, T_BLK], bf16, tag="ob")
                        nc.vector.tensor_copy(ob[:, :HB], ps[:, :HB])
                        nc.sync.dma_start(out4[b, h, :, :HB], ob[:, :HB])
                        nc.vector.tensor_copy(ob[:, HB:], ps[:, HB:])
                        nc.sync.dma_start(out4[b, h, :, HB:], ob[:, HB:])
                    else:
                        # Very last h-tile: run it as two independent N=256
                        # accumulation chains over token halves (separate
                        # PSUM banks) so the first half's copy+store overlap
                        # the second half's matmuls -- only 64KB drains after
                        # the final matmul instead of 128KB.
                        psA = ps2.tile([P, T_BLK], f32, tag="ps2t")
                        psB = ps2.tile([P, T_BLK], f32, tag="ps2t")
                        ob = opool.tile([P, T_BLK], bf16, tag="ob")
                        for f in range(FO):
                            nc.tensor.matmul(
                                psA[:, 0:HB],
                                w2_sb[:, h, f * P:(f + 1) * P],
                                it[:, f, 0:HB],
                                start=(f == 0),
                                stop=(f == FO - 1),
                            )
                        nc.vector.tensor_copy(ob[:, :HB], psA[:, 0:HB])
                        nc.sync.dma_start(out4[b, h, :, :HB], ob[:, :HB])
                        for f in range(FO):
                            nc.tensor.matmul(
                                psB[:, 0:HB],
                                w2_sb[:, h, f * P:(f + 1) * P],
                                it[:, f, HB:],
                                start=(f == 0),
                                stop=(f == FO - 1),
                            )
                        nc.vector.tensor_copy(ob[:, HB:], psB[:, 0:HB])
                        nc.sync.dma_start(out4[b, h, :, HB:], ob[:, HB:])

            # Stage the weight stream behind compute progress so the bulk of
            # the 16MB of weights never contends with the critical path:
            # w1 f-tile chunk c waits for the f-tile W1_LOOKAHEAD tiles ahead
            # of its first consumer; w2 chunk c is gated on the tail f-tiles
            # of GEMM1 block 0 (w2 is first read ~55us in).
            for c in range(W1_UNGATED, FO):
                add_dep_helper(
                    w1_dmas[c].ins, mm_first[(0, c - W1_LOOKAHEAD)].ins,
                    sync=True, reason="stage w1 load behind compute",
                )
            for c in range(NW2):
                add_dep_helper(
                    w2_dmas[c].ins, mm_first[(0, FO - NW2 - 6 + c)].ins,
                    sync=True, reason="stage w2 load behind compute",
                )
    nc.compile()
    return nc


def _get_nc():
    global _nc_cache
    if _nc_cache is None:
        _nc_cache = _build_nc()
    return _nc_cache


def kernel(x, w1, w2, rows_per_expert):
    global LAST_RESULTS
    from concourse.bass_utils import run_bass_kernel_spmd

    x = np.asarray(x)
    w1 = np.asarray(w1)
    w2 = np.asarray(w2)
    rpe = int(rows_per_expert)
    assert x.shape == (E * rpe, H) and rpe == T_PER_E
    assert w1.shape == (E, H, F) and w2.shape == (E, F, H)

    bf16 = ml_dtypes.bfloat16
    in_maps = []
    for e in range(E):
        xe = x[e * rpe:(e + 1) * rpe].astype(bf16)      # [T, H]
        # [b*T_BLK+ti, ho*128+p] -> [b, p, ho, ti]
        xpm = np.ascontiguousarray(
            xe.reshape(NBLK, T_BLK, HO, P).transpose(0, 3, 2, 1)
        )
        # w1[ho*128+p, f*128+fi] -> [f, p, ho*128+fi]
        w1m = np.ascontiguousarray(
            w1[e].astype(bf16).reshape(HO, P, FO, P).transpose(2, 1, 0, 3)
        ).reshape(FO, P, H)
        # w2[fo*128+p, h*128+hi] -> [h, p, fo*128+hi]
        w2m = np.ascontiguousarray(
            w2[e].astype(bf16).reshape(FO, P, HO, P).transpose(2, 1, 0, 3)
        ).reshape(HO, P, F)
        # boot pack: per partition [w1 chunk 0 | xb0 h0..h7]
        bootm = np.concatenate(
            [w1m[0], xpm[0].reshape(P, HO * T_BLK)], axis=1
        )
        bootm = np.ascontiguousarray(bootm)
        in_maps.append({"xp": xpm, "w1p": w1m, "w2p": w2m, "bootp": bootm})

    res = run_bass_kernel_spmd(_get_nc(), in_maps, list(range(E)), trace=TRACE)
    LAST_RESULTS = res

    out = np.empty((E * rpe, H), dtype=np.float32)
    for e in range(E):
        # [b, h, p, ti] -> [b*T_BLK+ti, h*128+p]
        o4 = res.results[e]["out4"].astype(np.float32)
        out[e * rpe:(e + 1) * rpe] = o4.transpose(0, 3, 1, 2).reshape(rpe, H)
    return out

